# revision 28
# baseline (speedup 1.0000x reference)
import sys

sys.path.insert(0, "/opt/trn_rl_repo")

import numpy as np
import ml_dtypes

import concourse.bass as bass
import concourse.tile as tile
from concourse import bacc, mybir
from concourse.bass_utils import run_bass_kernel_spmd
from concourse.masks import make_identity

BF16 = mybir.dt.bfloat16
F32 = mybir.dt.float32
U32 = mybir.dt.uint32
F8 = mybir.dt.float8e4
WSCALE = 64.0

N_CORES = 8
P = 128
WAY, SHOT, SEQ, TSS = 5, 5, 10, 2
T = 45                      # C(10,2) tuple pairs
NQ = 75                     # global queries
NQL = 10                    # queries per core (core 7: 5 real + 5 dup)
QROWS = NQL * T             # 450
QPAD = 512                  # 4 x 128
QT = 4                      # qrow tiles
SROWS = WAY * SHOT * T      # 1125
SPAD = 1152                 # 9 x 128
ST = 9                      # srow tiles
CB = SHOT * T               # 225 support cols per class
D = 2048                    # out dim
KT = 16                     # feat tiles (128 each)
FQ = NQL * SEQ              # 100 query frames
FS = WAY * SHOT * SEQ       # 250 support frames
F = FQ + FS                 # 350
NSPLITS = [(0, 384), (384, 768), (768, 1152)]
BIG = 1.0e18
CH = 4                      # phase-1 m-chunk for batched tuple expansion

# run lengths of PAIRS grouped by first index i: i pairs with j=i+1..9
RUNS = [(i, 9 - i) for i in range(SEQ - 1)]
TSTART = [0]
for _, r in RUNS[:-1]:
    TSTART.append(TSTART[-1] + r)

# s-s gram: 384-wide col blocks (r, cs) needed iff they touch the upper
# triangle (3*cs+2 >= r); lower 128-blocks not covered directly come from
# transposing their mirror.
SS_BLOCKS = [(r, cs) for r in range(ST) for cs in range(3)
             if 3 * cs + 2 >= r]
# lower 128-block (R, C), C<R, covered directly iff 3*(C//3)+2 >= R
NEED_T = [(r_, c_) for r_ in range(ST) for c_ in range(r_)
          if 3 * (c_ // 3) + 2 < r_]
# transposes are produced while computing block (c_, r_//3) (which contains
# upper 128-block (c_, r_)); map (src_r, src_cs) -> list of sub cols to
# transpose into their mirrored slots
TRANS_FROM = {}
for (r_, c_) in NEED_T:
    # source upper block row-tile c_, col 128-block r_
    src = (c_, r_ // 3)
    TRANS_FROM.setdefault(src, []).append(r_)


def _build():
    nc = bacc.Bacc(None, num_devices=N_CORES)

    # ---- I/O ----
    wt = nc.declare_dram_parameter("wt", [KT, P, 2 * KT * P], F8, isOutput=False)
    xt = nc.declare_dram_parameter("xt", [P, KT * F], F8, isOutput=False)
    bias = nc.declare_dram_parameter("bias", [P, KT], F32, isOutput=False)
    invb = nc.declare_dram_parameter("invb", [P, QT], F32, isOutput=False)
    gmat = nc.declare_dram_parameter("gmat", [P, QT * 16], F32, isOutput=False)
    out_v = nc.declare_dram_parameter("out_v", [16, 16], F32, isOutput=True)
    out_ms = nc.declare_dram_parameter("out_ms", [WAY, 1], F32, isOutput=True)

    # internal DRAM
    ss2_c = [nc.dram_tensor(f"ss2_{c}", [CB, SPAD], BF16)
             for c in range(WAY)]
    rec_in = nc.dram_tensor("rec_in", [WAY, SPAD], F32)
    rec_out = nc.dram_tensor("rec_out", [WAY, SPAD], F32, addr_space="Shared")

    with tile.TileContext(nc) as tc:
        with tc.tile_pool(name="persist", bufs=1) as pp:
            # ---- persistent SBUF ----
            sb_xt = pp.tile([P, KT * F], F8)
            nc.sync.dma_start(out=sb_xt[:], in_=xt[:])
            sb_bias = pp.tile([P, KT], F32)
            nc.sync.dma_start(out=sb_bias[:], in_=bias[:])
            sb_invb = pp.tile([P, QT], F32)
            nc.sync.dma_start(out=sb_invb[:], in_=invb[:])
            sb_gmat = pp.tile([P, QT * 16], F32)
            nc.sync.dma_start(out=sb_gmat[:], in_=gmat[:])

            ES = pp.tile([P, KT, SPAD], F8)
            EQ = pp.tile([P, KT, QPAD], F8)
            D2 = pp.tile([P, QT, SPAD], F32)

            ones_c = pp.tile([P, 1], BF16)
            nc.vector.memset(ones_c[:], 1.0)
            ones_m = pp.tile([P, P], BF16)
            nc.vector.memset(ones_m[:], 1.0)
            ident = pp.tile([P, P], BF16)
            make_identity(nc, ident[:])

            # norms: cols 0..3 = query-tile norms, 4..12 = support-tile norms
            qsn = pp.tile([P, 16], F32)
            snb = pp.tile([P, SPAD], F32)

            import os
            bisect = int(os.environ.get("KBISECT", "0"))
            _phase1(nc, tc, sb_xt, sb_bias, wt, ES, EQ, ones_c, ones_m,
                    qsn, snb)
            if bisect == 1:
                dbg = pp.tile([16, 16], F32)
                nc.vector.tensor_copy(dbg[:], EQ[:16, 0, :16])
                nc.sync.dma_start(out=out_v[:], in_=dbg[:])
            else:
                _phase2(nc, tc, ES, EQ, D2, qsn, snb, ss2_c, ident,
                        sb_invb, sb_gmat, ones_c, rec_in, rec_out,
                        out_v, out_ms, bisect)
                if bisect == 2:
                    dbg = pp.tile([16, 16], F32)
                    nc.vector.tensor_copy(dbg[:], D2[:16, 0, :16])
                    nc.sync.dma_start(out=out_v[:], in_=dbg[:])

    nc.compile()
    return nc


def _phase1(nc, tc, sb_xt, sb_bias, wt, ES, EQ, ones_c, ones_m, qsn, snb):
    with (
        tc.tile_pool(name="wpool", bufs=3) as wp,
        tc.tile_pool(name="stage", bufs=1) as st,
        tc.tile_pool(name="work", bufs=1) as wk,
        tc.tile_pool(name="ps_ab", bufs=2, space="PSUM") as ps_ab,
        tc.tile_pool(name="ps_n", bufs=1, space="PSUM") as ps_n,
    ):
        A_all = st.tile([P, KT, F], BF16)
        B_all = st.tile([P, KT, F], BF16)
        bias_h = st.tile([P, KT], F32)
        nc.vector.tensor_scalar(out=bias_h[:], in0=sb_bias[:], scalar1=0.5,
                                scalar2=None, op0=mybir.AluOpType.mult)
        # 13 norm columns accumulate in ONE psum bank: start=True zeroes the
        # whole 2KB zero-region (= the bank), so only the globally-first
        # matmul may set it; later columns' first writes land on
        # pending-zero bytes and read as 0.
        qsn_ps = ps_n.tile([P, 16], F32)
        snb_ps = [ps_n.tile([P, 384], F32, tag=f"snb{sp}", name=f"snb_ps{sp}")
                  for sp in range(3)]

        # zero padding columns once (they feed grams/norms; must be finite 0)
        nc.vector.memset(EQ[:, :, QROWS:], 0.0)
        nc.vector.memset(ES[:, :, SROWS:], 0.0)

        for m in range(KT):
            wm = wp.tile([P, 2 * KT * P], F8, tag="wm")
            nc.sync.dma_start(out=wm[:], in_=wt[m])
            psA = ps_ab.tile([P, F], F32, tag="psA")
            psB = ps_ab.tile([P, F], F32, tag="psB")
            wm_off = wm[:].offset
            wm_pd = wm[:].ap[0]
            xt_off = sb_xt[:].offset
            xt_pd = sb_xt[:].ap[0]
            KH = KT // 2
            for kk in range(KH):
                rhs = bass.AP(sb_xt.tensor, xt_off + 2 * kk * F,
                              [xt_pd, [F, 2], [1, F]])
                nc.tensor.matmul(
                    out=psA[:],
                    lhsT=bass.AP(wm.tensor, wm_off + 2 * kk * P,
                                 [wm_pd, [P, 2], [1, P]]),
                    rhs=rhs, perf_mode=mybir.MatmulPerfMode.DoubleRow,
                    start=(kk == 0), stop=(kk == KH - 1))
            for kk in range(KH):
                rhs = bass.AP(sb_xt.tensor, xt_off + 2 * kk * F,
                              [xt_pd, [F, 2], [1, F]])
                nc.tensor.matmul(
                    out=psB[:],
                    lhsT=bass.AP(wm.tensor, wm_off + (KT + 2 * kk) * P,
                                 [wm_pd, [P, 2], [1, P]]),
                    rhs=rhs, perf_mode=mybir.MatmulPerfMode.DoubleRow,
                    start=(kk == 0), stop=(kk == KH - 1))
            nc.scalar.activation(
                out=A_all[:, m, :], in_=psA[:],
                func=mybir.ActivationFunctionType.Identity,
                bias=bias_h[:, m:m + 1], scale=1.0 / WSCALE)
            nc.scalar.activation(
                out=B_all[:, m, :], in_=psB[:],
                func=mybir.ActivationFunctionType.Identity,
                bias=bias_h[:, m:m + 1], scale=1.0 / WSCALE)

            if m % CH != CH - 1:
                continue
            m0 = m - (CH - 1)
            # ---- batched tuple expansion for m-tiles m0..m ----
            tq = wk.tile([P, CH, QROWS], BF16, tag="tq")
            ts_ = wk.tile([P, CH, SROWS], BF16, tag="ts")
            a_off = A_all[:].offset
            b_off = B_all[:].offset
            pdim_a = A_all[:].ap[0]
            for (i, run), ts0 in zip(RUNS, TSTART):
                # queries: tuple row n*45+t <- A[m, n*10+i] + B[m, n*10+i+1+t]
                o = bass.AP(tq.tensor, tq[:].offset + ts0,
                            [tq[:].ap[0], [QROWS, CH], [T, NQL], [1, run]])
                a = bass.AP(A_all.tensor, a_off + m0 * F + i,
                            [pdim_a, [F, CH], [SEQ, NQL], [0, run]])
                b = bass.AP(B_all.tensor, b_off + m0 * F + i + 1,
                            [pdim_a, [F, CH], [SEQ, NQL], [1, run]])
                nc.gpsimd.tensor_add(out=o, in0=a, in1=b)
                # support frames start at col FQ
                o = bass.AP(ts_.tensor, ts_[:].offset + ts0,
                            [ts_[:].ap[0], [SROWS, CH], [T, WAY * SHOT], [1, run]])
                a = bass.AP(A_all.tensor, a_off + m0 * F + FQ + i,
                            [pdim_a, [F, CH], [SEQ, WAY * SHOT], [0, run]])
                b = bass.AP(B_all.tensor, b_off + m0 * F + FQ + i + 1,
                            [pdim_a, [F, CH], [SEQ, WAY * SHOT], [1, run]])
                nc.vector.tensor_add(out=o, in0=a, in1=b)

            eq_out = bass.AP(EQ.tensor, EQ[:].offset + m0 * QPAD,
                             [EQ[:].ap[0], [QPAD, CH], [1, QROWS]])
            nc.scalar.activation(out=eq_out, in_=tq[:],
                                 func=mybir.ActivationFunctionType.Relu)
            es_out = bass.AP(ES.tensor, ES[:].offset + m0 * SPAD,
                             [ES[:].ap[0], [SPAD, CH], [1, SROWS]])
            nc.scalar.activation(out=es_out, in_=ts_[:],
                                 func=mybir.ActivationFunctionType.Relu)

            # squares (bf16) for norms: relu(x)^2 = max(x,0)*x, all-bf16
            eq2 = wk.tile([P, CH, QPAD], BF16, tag="eq2")
            es2 = wk.tile([P, CH, SPAD], BF16, tag="es2")
            nc.vector.memset(
                bass.AP(eq2.tensor, eq2[:].offset + QROWS,
                        [eq2[:].ap[0], [QPAD, CH], [1, QPAD - QROWS]]), 0.0)
            nc.vector.memset(
                bass.AP(es2.tensor, es2[:].offset + SROWS,
                        [es2[:].ap[0], [SPAD, CH], [1, SPAD - SROWS]]), 0.0)
            nc.vector.scalar_tensor_tensor(
                out=bass.AP(eq2.tensor, eq2[:].offset,
                            [eq2[:].ap[0], [QPAD, CH], [1, QROWS]]),
                in0=tq[:], scalar=0.0, in1=tq[:],
                op0=mybir.AluOpType.max, op1=mybir.AluOpType.mult)
            nc.vector.scalar_tensor_tensor(
                out=bass.AP(es2.tensor, es2[:].offset,
                            [es2[:].ap[0], [SPAD, CH], [1, SROWS]]),
                in0=ts_[:], scalar=0.0, in1=ts_[:],
                op0=mybir.AluOpType.max, op1=mybir.AluOpType.mult)

            # norm partial sums accumulated in PSUM across all 16 m
            for mj in range(CH):
                m_ = m0 + mj
                for mq in range(QT):
                    nc.tensor.matmul(
                        out=qsn_ps[:, mq:mq + 1],
                        lhsT=eq2[:, mj, mq * P:(mq + 1) * P],
                        rhs=ones_c[:], start=(m_ == 0 and mq == 0),
                        stop=(m_ == KT - 1 and False))
                for ms in range(ST):
                    nc.tensor.matmul(
                        out=qsn_ps[:, QT + ms:QT + ms + 1],
                        lhsT=es2[:, mj, ms * P:(ms + 1) * P],
                        rhs=ones_c[:], start=False,
                        stop=(m_ == KT - 1 and ms == ST - 1))
                for sp3, (lo, hi) in enumerate(NSPLITS):
                    nc.tensor.matmul(
                        out=snb_ps[sp3][:], lhsT=ones_m[:],
                        rhs=es2[:, mj, lo:hi], start=(m_ == 0),
                        stop=(m_ == KT - 1))

        nc.vector.tensor_copy(out=qsn[:, :13], in_=qsn_ps[:, :13])
        for sp3, (lo, hi) in enumerate(NSPLITS):
            nc.scalar.copy(out=snb[:, lo:hi], in_=snb_ps[sp3][:])


def _phase2(nc, tc, ES, EQ, D2, qsn, snb, ss2_c, ident,
            sb_invb, sb_gmat, ones_c, rec_in, rec_out, out_v, out_ms,
            bisect=0):
    # ---- phase 2+3: grams, reductions, gather, record, contrast ----
    with tc.tile_pool(name="red", bufs=1) as rp:
        V = rp.tile([P, QT, 16], F32)
        nc.vector.memset(V[:], 0.0)
        ave2 = rp.tile([P, QT, WAY], F32)
        idxl = rp.tile([P, QT, WAY], U32)
        m8 = rp.tile([P, 8], F32)
        i8 = rp.tile([P, 8], U32)
        dcall = rp.tile([P, QT, WAY, SHOT], F32)
        ssg = rp.tile([P, QT, WAY, SPAD], BF16)
        Ds = rp.tile([P, QT, SPAD], BF16)
        cmp_sum = rp.tile([P, WAY, SPAD], BF16)
        rec_sb = rp.tile([8, SPAD], F32)

        with (
            tc.tile_pool(name="ps_gram", bufs=4, space="PSUM") as ps_g,
            tc.tile_pool(name="ps_tr", bufs=2, space="PSUM") as ps_t,
            tc.tile_pool(name="gwork", bufs=3) as gw,
        ):
            # query-to-support gram first: the phase-3 front depends on it
            for mq in range(QT):
                for lo, hi in NSPLITS:
                    pg = ps_g.tile([P, 384], F32, tag="pg")
                    eq_off = EQ[:].offset
                    eq_pd = EQ[:].ap[0]
                    es_off = ES[:].offset
                    es_pd = ES[:].ap[0]
                    for kk in range(KT // 2):
                        nc.tensor.matmul(
                            out=pg[:],
                            lhsT=bass.AP(EQ.tensor,
                                         eq_off + 2 * kk * QPAD + mq * P,
                                         [eq_pd, [QPAD, 2], [1, P]]),
                            rhs=bass.AP(ES.tensor, es_off + 2 * kk * SPAD + lo,
                                        [es_pd, [SPAD, 2], [1, hi - lo]]),
                            perf_mode=mybir.MatmulPerfMode.DoubleRow,
                            start=(kk == 0), stop=(kk == KT // 2 - 1))
                    tmp = gw.tile([P, 384], F32, tag="gtmp")
                    nc.vector.scalar_tensor_tensor(
                        out=tmp[:], in0=pg[:], scalar=-2.0, in1=snb[:, lo:hi],
                        op0=mybir.AluOpType.mult, op1=mybir.AluOpType.add)
                    nc.scalar.activation(
                        out=D2[:, mq, lo:hi], in_=tmp[:],
                        func=mybir.ActivationFunctionType.Relu,
                        bias=qsn[:, mq:mq + 1])

            # phase-3 front (DVE), emitted interleaved with the s-s gram
            # blocks below so the s-s stt is not stuck behind it in the
            # DVE queue
            front_ops = []

            def _front_cm(c, mq):
                def emit():
                    blk = D2[:, mq, c * CB:(c + 1) * CB]
                    nc.vector.max(m8[:], blk)
                    nc.vector.max_index(i8[:], m8[:], blk)
                    nc.vector.tensor_copy(idxl[:, mq, c:c + 1], i8[:, 0:1])
                    nc.vector.tensor_copy(V[:, mq, c:c + 1], m8[:, 0:1])
                return emit

            for c in range(WAY):
                for mq in range(QT):
                    front_ops.append(_front_cm(c, mq))

            def _front_tail():
                d_off = D2[:].offset
                d_pdim = D2[:].ap[0]
                for mq in range(QT):
                    g = bass.AP(D2.tensor, d_off + mq * SPAD,
                                [d_pdim, [CB, WAY], [1, SHOT], [SHOT, T]])
                    nc.vector.reduce_max(dcall[:, mq], g,
                                         axis=mybir.AxisListType.X)
                nc.scalar.sqrt(
                    out=bass.AP(dcall.tensor, dcall[:].offset,
                                [dcall[:].ap[0], [1, QT * WAY * SHOT]]),
                    in_=bass.AP(dcall.tensor, dcall[:].offset,
                                [dcall[:].ap[0], [1, QT * WAY * SHOT]]))
                nc.vector.reduce_sum(ave2[:], dcall[:],
                                     axis=mybir.AxisListType.X)
                for mq in range(QT):
                    nc.scalar.activation(
                        out=ave2[:, mq, :], in_=ave2[:, mq, :],
                        func=mybir.ActivationFunctionType.Square,
                        bias=sb_invb[:, mq:mq + 1], scale=1.0 / SHOT)
                nc.scalar.sqrt(
                    out=bass.AP(V.tensor, V[:].offset,
                                [V[:].ap[0], [16, QT], [1, WAY]]),
                    in_=bass.AP(V.tensor, V[:].offset,
                                [V[:].ap[0], [16, QT], [1, WAY]]))
                nc.scalar.sqrt(
                    out=bass.AP(Ds.tensor, Ds[:].offset,
                                [Ds[:].ap[0], [1, QT * SPAD]]),
                    in_=bass.AP(D2.tensor, D2[:].offset,
                                [D2[:].ap[0], [1, QT * SPAD]]))

            # support-to-support: upper-triangle 384-blocks + transposed mirrors
            for bi, (r, cs) in enumerate(SS_BLOCKS):
                for _ in range(2):
                    if front_ops:
                        front_ops.pop(0)()
                lo, hi = NSPLITS[cs]
                pg = ps_g.tile([P, 384], F32, tag="pg")
                es_off = ES[:].offset
                es_pd = ES[:].ap[0]
                for kk in range(KT // 2):
                    nc.tensor.matmul(
                        out=pg[:],
                        lhsT=bass.AP(ES.tensor, es_off + 2 * kk * SPAD + r * P,
                                     [es_pd, [SPAD, 2], [1, P]]),
                        rhs=bass.AP(ES.tensor, es_off + 2 * kk * SPAD + lo,
                                    [es_pd, [SPAD, 2], [1, hi - lo]]),
                        perf_mode=mybir.MatmulPerfMode.DoubleRow,
                        start=(kk == 0), stop=(kk == KT // 2 - 1))
                tmp = gw.tile([P, 384], F32, tag="gtmp")
                nc.vector.scalar_tensor_tensor(
                    out=tmp[:], in0=pg[:], scalar=-2.0, in1=snb[:, lo:hi],
                    op0=mybir.AluOpType.mult, op1=mybir.AluOpType.add)
                ssb = gw.tile([P, 384], BF16, tag="ssb")
                nc.scalar.activation(
                    out=ssb[:], in_=tmp[:],
                    func=mybir.ActivationFunctionType.Relu,
                    bias=qsn[:, QT + r:QT + r + 1])
                g0 = r * P
                a = 0
                while a < min(P, SROWS - g0):
                    cls = (g0 + a) // CB
                    b = min(P, (cls + 1) * CB - g0)
                    nc.scalar.dma_start(
                        out=ss2_c[cls][g0 + a - cls * CB:g0 + b - cls * CB, lo:hi],
                        in_=ssb[a:b, :])
                    a = b
                for csub in TRANS_FROM.get((r, cs), []):
                    sub = csub - 3 * cs
                    pt = ps_t.tile([P, P], BF16, tag="pt")
                    nc.tensor.transpose(pt[:], ssb[:, sub * P:(sub + 1) * P], ident[:])
                    sbT = gw.tile([P, P], BF16, tag="sbT")
                    nc.scalar.copy(out=sbT[:], in_=pt[:])
                    g0 = csub * P
                    a = 0
                    while a < min(P, SROWS - g0):
                        cls = (g0 + a) // CB
                        b = min(P, (cls + 1) * CB - g0)
                        nc.scalar.dma_start(
                            out=ss2_c[cls][g0 + a - cls * CB:g0 + b - cls * CB,
                                           r * P:(r + 1) * P],
                            in_=sbT[a:b, :])
                        a = b

            while front_ops:
                front_ops.pop(0)()
            _front_tail()

        if bisect == 2:
            return

        with (
            tc.tile_pool(name="gath", bufs=2) as gp,
            tc.tile_pool(name="ps3", bufs=2, space="PSUM") as ps3,
            tc.tile_pool(name="dram3", bufs=1, space="DRAM") as dr3,
        ):
            # gathers: one argmax row per partition per (class, q-tile);
            # class-ascending so each starts once its ss2_c writes land
            for c in range(WAY):
                for mq in range(QT):
                    nc.gpsimd.indirect_dma_start(
                        out=ssg[:, mq, c, :], out_offset=None,
                        in_=ss2_c[c][:],
                        in_offset=bass.IndirectOffsetOnAxis(
                            ap=idxl[:, mq, c:c + 1], axis=0))

            if bisect == 3:
                dbg = rp.tile([16, 16], F32)
                nc.vector.memset(dbg[:], 0.0)
                nc.vector.tensor_copy(dbg[:, :WAY], ave2[:16, 0, :])
                nc.sync.dma_start(out=out_v[:], in_=dbg[:])
                return

            # transpose sqrt-distances now: the PE is idle while gathers and
            # cmp/record run; DsT[s-part, qcol] turns the masked sums into
            # tiny matmuls after the collective
            DsT = rp.tile([P, ST, QPAD], BF16)
            for ms in range(ST):
                for mq in range(QT):
                    ptd = ps3.tile([P, P], BF16, tag="ptd")
                    nc.tensor.transpose(
                        ptd[:], Ds[:, mq, ms * P:(ms + 1) * P], ident[:])
                    nc.scalar.copy(out=DsT[:, ms, mq * P:(mq + 1) * P],
                                   in_=ptd[:])

            # cmp in place (DVE), own-class zero + q-tile sums (gpsimd)
            for c in range(WAY):
                for mq in range(QT):
                    nc.vector.tensor_scalar(
                        out=ssg[:, mq, c, :], in0=ssg[:, mq, c, :],
                        scalar1=ave2[:, mq, c:c + 1], scalar2=None,
                        op0=mybir.AluOpType.is_gt)
                nc.vector.memset(ssg[:, :, c, c * CB:(c + 1) * CB], 0.0)
                nc.vector.tensor_add(out=cmp_sum[:, c, :], in0=ssg[:, 0, c, :],
                                     in1=ssg[:, 1, c, :])
                nc.vector.tensor_add(out=cmp_sum[:, c, :], in0=cmp_sum[:, c, :],
                                     in1=ssg[:, 2, c, :])
                nc.vector.tensor_add(out=cmp_sum[:, c, :], in0=cmp_sum[:, c, :],
                                     in1=ssg[:, 3, c, :])
                for lo, hi in NSPLITS:
                    pr = ps3.tile([1, 384], F32, tag="pr")
                    nc.tensor.matmul(out=pr[:], lhsT=ones_c[:],
                                     rhs=cmp_sum[:, c, lo:hi], start=True, stop=True)
                    prs = gp.tile([1, 384], F32, tag="prs")
                    nc.scalar.copy(out=prs[:], in_=pr[:])
                    nc.sync.dma_start(out=rec_sb[c:c + 1, lo:hi], in_=prs[:])

            if bisect == 4:
                dbg = rp.tile([16, 16], F32)
                nc.vector.memset(dbg[:], 0.0)
                nc.vector.tensor_copy(dbg[:WAY, :], rec_sb[:WAY, :16])
                nc.sync.dma_start(out=out_v[:], in_=dbg[:])
                return

            # global sum of record counts
            ci = dr3.tile([WAY, SPAD], F32)
            nc.sync.dma_start(out=ci[:], in_=rec_sb[:WAY, :])
            co = dr3.tile([WAY, SPAD], F32, addr_space="Shared")
            nc.gpsimd.collective_compute(
                "AllReduce", mybir.AluOpType.add,
                replica_groups=[list(range(N_CORES))],
                ins=[ci[:]], outs=[co[:]])
            recg = rp.tile([8, SPAD], F32)
            nc.sync.dma_start(out=recg[:WAY, :], in_=co[:])

            if bisect == 5:
                dbg = rp.tile([16, 16], F32)
                nc.vector.memset(dbg[:], 0.0)
                nc.vector.tensor_copy(dbg[:WAY, :], recg[:WAY, :16])
                nc.sync.dma_start(out=out_v[:], in_=dbg[:])
                return

            # thr per (class, block); mask; msum
            rsum = rp.tile([8, WAY], F32)
            nzc = rp.tile([8, SPAD], F32)
            nz = rp.tile([8, WAY], F32)
            thr = rp.tile([8, WAY], F32)
            g = recg[:WAY, :SROWS].rearrange("p (b s) -> p b s", b=WAY)
            nc.vector.reduce_sum(rsum[:WAY, :], g, axis=mybir.AxisListType.X)
            nc.vector.tensor_scalar(
                out=nzc[:WAY, :SROWS], in0=recg[:WAY, :SROWS],
                scalar1=0.5, scalar2=None, op0=mybir.AluOpType.is_gt)
            nc.vector.reduce_sum(
                nz[:WAY, :], nzc[:WAY, :SROWS].rearrange("p (b s) -> p b s", b=WAY),
                axis=mybir.AxisListType.X)
            nc.vector.tensor_scalar(
                out=nz[:WAY, :], in0=nz[:WAY, :], scalar1=1.0, scalar2=None,
                op0=mybir.AluOpType.max)
            nc.vector.reciprocal(out=nz[:WAY, :], in_=nz[:WAY, :])
            nc.vector.tensor_mul(out=thr[:WAY, :], in0=rsum[:WAY, :], in1=nz[:WAY, :])

            mask = rp.tile([8, SPAD], BF16)
            nc.vector.memset(mask[:], 0.0)
            for b in range(WAY):
                nc.vector.tensor_scalar(
                    out=mask[:WAY, b * CB:(b + 1) * CB],
                    in0=recg[:WAY, b * CB:(b + 1) * CB],
                    scalar1=thr[:WAY, b:b + 1], scalar2=None,
                    op0=mybir.AluOpType.is_lt)
            msum = rp.tile([8, 1], F32)
            nc.vector.reduce_sum(msum[:WAY, :], mask[:WAY, :SROWS], axis=mybir.AxisListType.X)
            nc.sync.dma_start(out=out_ms[:], in_=msum[:WAY, :])
            if bisect == 6:
                dbg = rp.tile([16, 16], F32)
                nc.vector.memset(dbg[:], 0.0)
                nc.vector.tensor_copy(dbg[:WAY, :], mask[:WAY, :16])
                nc.sync.dma_start(out=out_v[:], in_=dbg[:])
                return

            # masked mean distances: V[:, mq, 5+c] = sum_s DsT[s, q] mask[c, s]
            # mask rows transposed to [s-part, class]; contraction over s via PE
            maskT = rp.tile([P, ST, 8], BF16)
            for ms in range(ST):
                ptm = ps3.tile([P, 8], BF16, tag="ptm", bufs=1)
                nc.tensor.transpose(
                    ptm[:], mask[:, ms * P:(ms + 1) * P], ident[:8, :8])
                nc.scalar.copy(out=maskT[:, ms, :], in_=ptm[:])
            for mq in range(QT):
                pms = ps3.tile([P, 8], F32, tag="pms", bufs=1)
                for ms in range(ST):
                    nc.tensor.matmul(
                        out=pms[:], lhsT=DsT[:, ms, mq * P:(mq + 1) * P],
                        rhs=maskT[:, ms, :], start=(ms == 0), stop=(ms == ST - 1))
                nc.scalar.copy(out=V[:, mq, WAY:WAY + WAY], in_=pms[:, :WAY])

            # group rows by query: out[q, col] = sum_r gmat[r, q] * V[r, col]
            if bisect == 7:
                dbg = rp.tile([16, 16], F32)
                nc.vector.tensor_copy(dbg[:], V[:16, 0, :])
                nc.sync.dma_start(out=out_v[:], in_=dbg[:])
                return
            pv = ps3.tile([16, 16], F32, tag="pv", bufs=1)
            for t in range(QT):
                nc.tensor.matmul(
                    out=pv[:], lhsT=sb_gmat[:, t * 16:(t + 1) * 16], rhs=V[:, t, :],
                    start=(t == 0), stop=(t == QT - 1))
            vsb = rp.tile([16, 16], F32)
            nc.scalar.copy(out=vsb[:], in_=pv[:])
            nc.sync.dma_start(out=out_v[:], in_=vsb[:])


_CACHE = {}


def _get_nc():
    if "nc" not in _CACHE:
        _CACHE["nc"] = _build()
    return _CACHE["nc"]


def kernel(support_set, support_labels, queries, clsW_w, clsW_b):
    support_set = np.asarray(support_set, dtype=np.float32)
    queries = np.asarray(queries, dtype=np.float32)
    W = np.asarray(clsW_w, dtype=np.float32)
    b = np.asarray(clsW_b, dtype=np.float32)
    bf = ml_dtypes.bfloat16
    f8 = ml_dtypes.float8_e4m3

    # weight tiles: wt[m, kp, h, ko, mf] = W[m*128+mf, h*2048+ko*128+kp]
    wt = np.ascontiguousarray(
        (W * WSCALE).reshape(KT, P, 2, KT, P).transpose(0, 4, 2, 3, 1)).astype(f8)
    bias_h = np.ascontiguousarray(b.reshape(KT, P).T)

    # per-core frame matrices
    sup_frames = support_set.reshape(FS, D)
    in_maps = []
    rows = np.arange(QPAD)
    gm = np.zeros((QPAD, 16), np.float32)
    valid_all = rows < QROWS
    gm[valid_all, (rows // T)[valid_all].clip(0, 15)] = 1.0
    gmat_h = np.ascontiguousarray(gm.reshape(QT, P, 16).transpose(1, 0, 2))
    for k in range(N_CORES):
        qids = [(10 * k + j) % NQ for j in range(NQL)]
        qf = queries[qids].reshape(FQ, D)
        frames = np.concatenate([qf, sup_frames], axis=0)  # [350, 2048]
        xth = np.ascontiguousarray(
            frames.T.reshape(KT, P, F).transpose(1, 0, 2)).astype(f8)
        inv = np.where(rows < (QROWS if k < N_CORES - 1 else 5 * T), 0.0, BIG)
        inv = inv.astype(np.float32)
        invb_h = np.ascontiguousarray(inv.reshape(QT, P).T)
        in_maps.append({
            "wt": wt, "xt": xth, "bias": bias_h,
            "invb": invb_h, "gmat": gmat_h,
        })

    nc = _get_nc()
    _CACHE["last_in_maps"] = in_maps
    res = run_bass_kernel_spmd(nc, in_maps, list(range(N_CORES))).results

    msum = np.maximum(res[0]["out_ms"].reshape(WAY), 1.0)
    dist_max = np.zeros((NQ, WAY), np.float32)
    md_raw = np.zeros((NQ, WAY), np.float32)
    for k in range(N_CORES):
        v = res[k]["out_v"]
        for j in range(NQL):
            q = 10 * k + j
            if q >= NQ:
                break
            dist_max[q] = v[j, :WAY] / T
            md_raw[q] = v[j, WAY:2 * WAY]
    contrast = md_raw / (T * (WAY - 1) * msum[None, :])
    logits = dist_max / (contrast + dist_max)
    return dist_max, logits


# revision 29
# speedup vs baseline: 1.0535x; 1.0535x over previous
import sys

sys.path.insert(0, "/opt/trn_rl_repo")

import numpy as np
import ml_dtypes

import concourse.bass as bass
import concourse.tile as tile
from concourse import bacc, mybir
from concourse.bass_utils import run_bass_kernel_spmd
from concourse.masks import make_identity

BF16 = mybir.dt.bfloat16
F32 = mybir.dt.float32
U32 = mybir.dt.uint32
F8 = mybir.dt.float8e4
WSCALE = 64.0

N_CORES = 8
P = 128
WAY, SHOT, SEQ, TSS = 5, 5, 10, 2
T = 45                      # C(10,2) tuple pairs
NQ = 75                     # global queries
NQL = 10                    # queries per core (core 7: 5 real + 5 dup)
QROWS = NQL * T             # 450
QPAD = 512                  # 4 x 128
QT = 4                      # qrow tiles
SROWS = WAY * SHOT * T      # 1125
SPAD = 1152                 # 9 x 128
ST = 9                      # srow tiles
CB = SHOT * T               # 225 support cols per class
D = 2048                    # out dim
KT = 16                     # feat tiles (128 each)
FQ = NQL * SEQ              # 100 query frames
FS = WAY * SHOT * SEQ       # 250 support frames
F = FQ + FS                 # 350
NSPLITS = [(0, 384), (384, 768), (768, 1152)]
BIG = 1.0e18
CH = 4                      # phase-1 m-chunk for batched tuple expansion

# run lengths of PAIRS grouped by first index i: i pairs with j=i+1..9
RUNS = [(i, 9 - i) for i in range(SEQ - 1)]
TSTART = [0]
for _, r in RUNS[:-1]:
    TSTART.append(TSTART[-1] + r)

# s-s gram: 384-wide col blocks (r, cs) needed iff they touch the upper
# triangle (3*cs+2 >= r); lower 128-blocks not covered directly come from
# transposing their mirror.
SS_BLOCKS = [(r, cs) for r in range(ST) for cs in range(3)
             if 3 * cs + 2 >= r]
# lower 128-block (R, C), C<R, covered directly iff 3*(C//3)+2 >= R
NEED_T = [(r_, c_) for r_ in range(ST) for c_ in range(r_)
          if 3 * (c_ // 3) + 2 < r_]
# transposes are produced while computing block (c_, r_//3) (which contains
# upper 128-block (c_, r_)); map (src_r, src_cs) -> list of sub cols to
# transpose into their mirrored slots
TRANS_FROM = {}
for (r_, c_) in NEED_T:
    # source upper block row-tile c_, col 128-block r_
    src = (c_, r_ // 3)
    TRANS_FROM.setdefault(src, []).append(r_)


def _build():
    nc = bacc.Bacc(None, num_devices=N_CORES)

    # ---- I/O ----
    wt = nc.declare_dram_parameter("wt", [KT, P, 2 * KT * P], F8, isOutput=False)
    xt = nc.declare_dram_parameter("xt", [P, KT * F], F8, isOutput=False)
    bias = nc.declare_dram_parameter("bias", [P, KT], F32, isOutput=False)
    invb = nc.declare_dram_parameter("invb", [P, QT], F32, isOutput=False)
    gmat = nc.declare_dram_parameter("gmat", [P, QT * 16], F32, isOutput=False)
    out_v = nc.declare_dram_parameter("out_v", [16, 16], F32, isOutput=True)
    out_ms = nc.declare_dram_parameter("out_ms", [WAY, 1], F32, isOutput=True)

    # internal DRAM
    ss2_c = [nc.dram_tensor(f"ss2_{c}", [CB, SPAD], BF16)
             for c in range(WAY)]
    rec_in = nc.dram_tensor("rec_in", [WAY, SPAD], F32)
    rec_out = nc.dram_tensor("rec_out", [WAY, SPAD], F32, addr_space="Shared")

    with tile.TileContext(nc) as tc:
        with tc.tile_pool(name="persist", bufs=1) as pp:
            # ---- persistent SBUF ----
            sb_xt = pp.tile([P, KT * F], F8)
            nc.sync.dma_start(out=sb_xt[:], in_=xt[:])
            sb_bias = pp.tile([P, KT], F32)
            nc.sync.dma_start(out=sb_bias[:], in_=bias[:])
            sb_invb = pp.tile([P, QT], F32)
            nc.sync.dma_start(out=sb_invb[:], in_=invb[:])
            sb_gmat = pp.tile([P, QT * 16], F32)
            nc.sync.dma_start(out=sb_gmat[:], in_=gmat[:])

            ES = pp.tile([P, KT, SPAD], F8)
            EQ = pp.tile([P, KT, QPAD], F8)
            D2 = pp.tile([P, QT, SPAD], F32)

            ones_c = pp.tile([P, 1], BF16)
            nc.vector.memset(ones_c[:], 1.0)
            ones_m = pp.tile([P, P], BF16)
            nc.vector.memset(ones_m[:], 1.0)
            ident = pp.tile([P, P], BF16)
            make_identity(nc, ident[:])

            # norms: cols 0..3 = query-tile norms, 4..12 = support-tile norms
            qsn = pp.tile([P, 16], F32)
            snb = pp.tile([P, SPAD], F32)

            import os
            bisect = int(os.environ.get("KBISECT", "0"))
            _phase1(nc, tc, sb_xt, sb_bias, wt, ES, EQ, ones_c, ones_m,
                    qsn, snb)
            if bisect == 1:
                dbg = pp.tile([16, 16], F32)
                nc.vector.tensor_copy(dbg[:], EQ[:16, 0, :16])
                nc.sync.dma_start(out=out_v[:], in_=dbg[:])
            else:
                _phase2(nc, tc, ES, EQ, D2, qsn, snb, ss2_c, ident,
                        sb_invb, sb_gmat, ones_c, rec_in, rec_out,
                        out_v, out_ms, bisect)
                if bisect == 2:
                    dbg = pp.tile([16, 16], F32)
                    nc.vector.tensor_copy(dbg[:], D2[:16, 0, :16])
                    nc.sync.dma_start(out=out_v[:], in_=dbg[:])

    nc.compile()
    return nc


def _phase1(nc, tc, sb_xt, sb_bias, wt, ES, EQ, ones_c, ones_m, qsn, snb):
    with (
        tc.tile_pool(name="wpool", bufs=3) as wp,
        tc.tile_pool(name="stage", bufs=1) as st,
        tc.tile_pool(name="work", bufs=1) as wk,
        tc.tile_pool(name="ps_ab", bufs=2, space="PSUM") as ps_ab,
        tc.tile_pool(name="ps_n", bufs=1, space="PSUM") as ps_n,
    ):
        A_all = st.tile([P, KT, F], BF16)
        B_all = st.tile([P, KT, F], BF16)
        bias_h = st.tile([P, KT], F32)
        nc.vector.tensor_scalar(out=bias_h[:], in0=sb_bias[:], scalar1=0.5,
                                scalar2=None, op0=mybir.AluOpType.mult)
        # 13 norm columns accumulate in ONE psum bank: start=True zeroes the
        # whole 2KB zero-region (= the bank), so only the globally-first
        # matmul may set it; later columns' first writes land on
        # pending-zero bytes and read as 0.
        qsn_ps = ps_n.tile([P, 16], F32)
        snb_ps = [ps_n.tile([P, 384], F32, tag=f"snb{sp}", name=f"snb_ps{sp}")
                  for sp in range(3)]

        # zero padding columns once (they feed grams/norms; must be finite 0)
        nc.vector.memset(EQ[:, :, QROWS:], 0.0)
        nc.vector.memset(ES[:, :, SROWS:], 0.0)

        for m in range(KT):
            wm = wp.tile([P, 2 * KT * P], F8, tag="wm")
            nc.sync.dma_start(out=wm[:], in_=wt[m])
            psA = ps_ab.tile([P, F], F32, tag="psA")
            psB = ps_ab.tile([P, F], F32, tag="psB")
            wm_off = wm[:].offset
            wm_pd = wm[:].ap[0]
            xt_off = sb_xt[:].offset
            xt_pd = sb_xt[:].ap[0]
            KH = KT // 2
            for kk in range(KH):
                rhs = bass.AP(sb_xt.tensor, xt_off + 2 * kk * F,
                              [xt_pd, [F, 2], [1, F]])
                nc.tensor.matmul(
                    out=psA[:],
                    lhsT=bass.AP(wm.tensor, wm_off + 2 * kk * P,
                                 [wm_pd, [P, 2], [1, P]]),
                    rhs=rhs, perf_mode=mybir.MatmulPerfMode.DoubleRow,
                    start=(kk == 0), stop=(kk == KH - 1))
            for kk in range(KH):
                rhs = bass.AP(sb_xt.tensor, xt_off + 2 * kk * F,
                              [xt_pd, [F, 2], [1, F]])
                nc.tensor.matmul(
                    out=psB[:],
                    lhsT=bass.AP(wm.tensor, wm_off + (KT + 2 * kk) * P,
                                 [wm_pd, [P, 2], [1, P]]),
                    rhs=rhs, perf_mode=mybir.MatmulPerfMode.DoubleRow,
                    start=(kk == 0), stop=(kk == KH - 1))
            nc.scalar.activation(
                out=A_all[:, m, :], in_=psA[:],
                func=mybir.ActivationFunctionType.Identity,
                bias=bias_h[:, m:m + 1], scale=1.0 / WSCALE)
            nc.scalar.activation(
                out=B_all[:, m, :], in_=psB[:],
                func=mybir.ActivationFunctionType.Identity,
                bias=bias_h[:, m:m + 1], scale=1.0 / WSCALE)

            if m % CH != CH - 1:
                continue
            m0 = m - (CH - 1)
            # ---- batched tuple expansion for m-tiles m0..m ----
            tq = wk.tile([P, CH, QROWS], BF16, tag="tq")
            ts_ = wk.tile([P, CH, SROWS], BF16, tag="ts")
            a_off = A_all[:].offset
            b_off = B_all[:].offset
            pdim_a = A_all[:].ap[0]
            for (i, run), ts0 in zip(RUNS, TSTART):
                # queries: tuple row n*45+t <- A[m, n*10+i] + B[m, n*10+i+1+t]
                o = bass.AP(tq.tensor, tq[:].offset + ts0,
                            [tq[:].ap[0], [QROWS, CH], [T, NQL], [1, run]])
                a = bass.AP(A_all.tensor, a_off + m0 * F + i,
                            [pdim_a, [F, CH], [SEQ, NQL], [0, run]])
                b = bass.AP(B_all.tensor, b_off + m0 * F + i + 1,
                            [pdim_a, [F, CH], [SEQ, NQL], [1, run]])
                nc.gpsimd.tensor_add(out=o, in0=a, in1=b)
                # support frames start at col FQ
                o = bass.AP(ts_.tensor, ts_[:].offset + ts0,
                            [ts_[:].ap[0], [SROWS, CH], [T, WAY * SHOT], [1, run]])
                a = bass.AP(A_all.tensor, a_off + m0 * F + FQ + i,
                            [pdim_a, [F, CH], [SEQ, WAY * SHOT], [0, run]])
                b = bass.AP(B_all.tensor, b_off + m0 * F + FQ + i + 1,
                            [pdim_a, [F, CH], [SEQ, WAY * SHOT], [1, run]])
                nc.vector.tensor_add(out=o, in0=a, in1=b)

            eq_out = bass.AP(EQ.tensor, EQ[:].offset + m0 * QPAD,
                             [EQ[:].ap[0], [QPAD, CH], [1, QROWS]])
            nc.scalar.activation(out=eq_out, in_=tq[:],
                                 func=mybir.ActivationFunctionType.Relu)
            es_out = bass.AP(ES.tensor, ES[:].offset + m0 * SPAD,
                             [ES[:].ap[0], [SPAD, CH], [1, SROWS]])
            nc.scalar.activation(out=es_out, in_=ts_[:],
                                 func=mybir.ActivationFunctionType.Relu)

            # squares (bf16) for norms: relu(x)^2 = max(x,0)*x, all-bf16
            eq2 = wk.tile([P, CH, QPAD], BF16, tag="eq2")
            es2 = wk.tile([P, CH, SPAD], BF16, tag="es2")
            nc.vector.memset(
                bass.AP(eq2.tensor, eq2[:].offset + QROWS,
                        [eq2[:].ap[0], [QPAD, CH], [1, QPAD - QROWS]]), 0.0)
            nc.vector.memset(
                bass.AP(es2.tensor, es2[:].offset + SROWS,
                        [es2[:].ap[0], [SPAD, CH], [1, SPAD - SROWS]]), 0.0)
            nc.vector.scalar_tensor_tensor(
                out=bass.AP(eq2.tensor, eq2[:].offset,
                            [eq2[:].ap[0], [QPAD, CH], [1, QROWS]]),
                in0=tq[:], scalar=0.0, in1=tq[:],
                op0=mybir.AluOpType.max, op1=mybir.AluOpType.mult)
            nc.vector.scalar_tensor_tensor(
                out=bass.AP(es2.tensor, es2[:].offset,
                            [es2[:].ap[0], [SPAD, CH], [1, SROWS]]),
                in0=ts_[:], scalar=0.0, in1=ts_[:],
                op0=mybir.AluOpType.max, op1=mybir.AluOpType.mult)

            # norm partial sums accumulated in PSUM across all 16 m
            for mj in range(CH):
                m_ = m0 + mj
                for mq in range(QT):
                    nc.tensor.matmul(
                        out=qsn_ps[:, mq:mq + 1],
                        lhsT=eq2[:, mj, mq * P:(mq + 1) * P],
                        rhs=ones_c[:], start=(m_ == 0 and mq == 0),
                        stop=(m_ == KT - 1 and False))
                for ms in range(ST):
                    nc.tensor.matmul(
                        out=qsn_ps[:, QT + ms:QT + ms + 1],
                        lhsT=es2[:, mj, ms * P:(ms + 1) * P],
                        rhs=ones_c[:], start=False,
                        stop=(m_ == KT - 1 and ms == ST - 1))
                for sp3, (lo, hi) in enumerate(NSPLITS):
                    nc.tensor.matmul(
                        out=snb_ps[sp3][:], lhsT=ones_m[:],
                        rhs=es2[:, mj, lo:hi], start=(m_ == 0),
                        stop=(m_ == KT - 1))

        nc.vector.tensor_copy(out=qsn[:, :13], in_=qsn_ps[:, :13])
        for sp3, (lo, hi) in enumerate(NSPLITS):
            nc.scalar.copy(out=snb[:, lo:hi], in_=snb_ps[sp3][:])


def _phase2(nc, tc, ES, EQ, D2, qsn, snb, ss2_c, ident,
            sb_invb, sb_gmat, ones_c, rec_in, rec_out, out_v, out_ms,
            bisect=0):
    # ---- phase 2+3: grams, reductions, gather, record, contrast ----
    with tc.tile_pool(name="red", bufs=1) as rp:
        V = rp.tile([P, QT, 16], F32)
        nc.vector.memset(V[:], 0.0)
        ave2 = rp.tile([P, QT, WAY], F32)
        idxl = rp.tile([P, QT, WAY], U32)
        m8 = rp.tile([P, 8], F32)
        i8 = rp.tile([P, 8], U32)
        dcall = rp.tile([P, QT, WAY, SHOT], F32)
        ssg = rp.tile([P, QT, WAY, SPAD], BF16)
        Ds = rp.tile([P, QT, SPAD], BF16)
        cmp_sum = rp.tile([P, WAY, SPAD], BF16)
        rec_sb = rp.tile([8, SPAD], F32)

        with (
            tc.tile_pool(name="ps_gram", bufs=4, space="PSUM") as ps_g,
            tc.tile_pool(name="ps_tr", bufs=2, space="PSUM") as ps_t,
            tc.tile_pool(name="gwork", bufs=3) as gw,
        ):
            # query-to-support gram first: the phase-3 front depends on it
            for mq in range(QT):
                for lo, hi in NSPLITS:
                    pg = ps_g.tile([P, 384], F32, tag="pg")
                    eq_off = EQ[:].offset
                    eq_pd = EQ[:].ap[0]
                    es_off = ES[:].offset
                    es_pd = ES[:].ap[0]
                    for kk in range(KT // 2):
                        nc.tensor.matmul(
                            out=pg[:],
                            lhsT=bass.AP(EQ.tensor,
                                         eq_off + 2 * kk * QPAD + mq * P,
                                         [eq_pd, [QPAD, 2], [1, P]]),
                            rhs=bass.AP(ES.tensor, es_off + 2 * kk * SPAD + lo,
                                        [es_pd, [SPAD, 2], [1, hi - lo]]),
                            perf_mode=mybir.MatmulPerfMode.DoubleRow,
                            start=(kk == 0), stop=(kk == KT // 2 - 1))
                    tmp = gw.tile([P, 384], F32, tag="gtmp")
                    nc.vector.scalar_tensor_tensor(
                        out=tmp[:], in0=pg[:], scalar=-2.0, in1=snb[:, lo:hi],
                        op0=mybir.AluOpType.mult, op1=mybir.AluOpType.add)
                    nc.scalar.activation(
                        out=D2[:, mq, lo:hi], in_=tmp[:],
                        func=mybir.ActivationFunctionType.Relu,
                        bias=qsn[:, mq:mq + 1])

            # phase-3 front (DVE), emitted interleaved with the s-s gram
            # blocks below so the s-s stt is not stuck behind it in the
            # DVE queue
            front_ops = []

            def _front_cm(c, mq):
                def emit():
                    blk = D2[:, mq, c * CB:(c + 1) * CB]
                    nc.vector.max(m8[:], blk)
                    nc.vector.max_index(i8[:], m8[:], blk)
                    nc.vector.tensor_copy(idxl[:, mq, c:c + 1], i8[:, 0:1])
                    nc.vector.tensor_copy(V[:, mq, c:c + 1], m8[:, 0:1])
                return emit

            for c in range(WAY):
                for mq in range(QT):
                    front_ops.append(_front_cm(c, mq))

            def _front_tail():
                d_off = D2[:].offset
                d_pdim = D2[:].ap[0]
                for mq in range(QT):
                    g = bass.AP(D2.tensor, d_off + mq * SPAD,
                                [d_pdim, [CB, WAY], [1, SHOT], [SHOT, T]])
                    nc.vector.reduce_max(dcall[:, mq], g,
                                         axis=mybir.AxisListType.X)
                nc.scalar.sqrt(
                    out=bass.AP(dcall.tensor, dcall[:].offset,
                                [dcall[:].ap[0], [1, QT * WAY * SHOT]]),
                    in_=bass.AP(dcall.tensor, dcall[:].offset,
                                [dcall[:].ap[0], [1, QT * WAY * SHOT]]))
                nc.vector.reduce_sum(ave2[:], dcall[:],
                                     axis=mybir.AxisListType.X)
                for mq in range(QT):
                    nc.scalar.activation(
                        out=ave2[:, mq, :], in_=ave2[:, mq, :],
                        func=mybir.ActivationFunctionType.Square,
                        bias=sb_invb[:, mq:mq + 1], scale=1.0 / SHOT)
                nc.scalar.sqrt(
                    out=bass.AP(V.tensor, V[:].offset,
                                [V[:].ap[0], [16, QT], [1, WAY]]),
                    in_=bass.AP(V.tensor, V[:].offset,
                                [V[:].ap[0], [16, QT], [1, WAY]]))
                nc.scalar.sqrt(
                    out=bass.AP(Ds.tensor, Ds[:].offset,
                                [Ds[:].ap[0], [1, QT * SPAD]]),
                    in_=bass.AP(D2.tensor, D2[:].offset,
                                [D2[:].ap[0], [1, QT * SPAD]]))

            front_ops.append(_front_tail)

            # support-to-support: upper-triangle 384-blocks + transposed mirrors
            for bi, (r, cs) in enumerate(SS_BLOCKS):
                for _ in range(2):
                    if front_ops:
                        front_ops.pop(0)()
                lo, hi = NSPLITS[cs]
                pg = ps_g.tile([P, 384], F32, tag="pg")
                es_off = ES[:].offset
                es_pd = ES[:].ap[0]
                for kk in range(KT // 2):
                    nc.tensor.matmul(
                        out=pg[:],
                        lhsT=bass.AP(ES.tensor, es_off + 2 * kk * SPAD + r * P,
                                     [es_pd, [SPAD, 2], [1, P]]),
                        rhs=bass.AP(ES.tensor, es_off + 2 * kk * SPAD + lo,
                                    [es_pd, [SPAD, 2], [1, hi - lo]]),
                        perf_mode=mybir.MatmulPerfMode.DoubleRow,
                        start=(kk == 0), stop=(kk == KT // 2 - 1))
                tmp = gw.tile([P, 384], F32, tag="gtmp")
                nc.vector.scalar_tensor_tensor(
                    out=tmp[:], in0=pg[:], scalar=-2.0, in1=snb[:, lo:hi],
                    op0=mybir.AluOpType.mult, op1=mybir.AluOpType.add)
                ssb = gw.tile([P, 384], BF16, tag="ssb")
                nc.scalar.activation(
                    out=ssb[:], in_=tmp[:],
                    func=mybir.ActivationFunctionType.Relu,
                    bias=qsn[:, QT + r:QT + r + 1])
                g0 = r * P
                a = 0
                while a < min(P, SROWS - g0):
                    cls = (g0 + a) // CB
                    b = min(P, (cls + 1) * CB - g0)
                    nc.scalar.dma_start(
                        out=ss2_c[cls][g0 + a - cls * CB:g0 + b - cls * CB, lo:hi],
                        in_=ssb[a:b, :])
                    a = b
                for csub in TRANS_FROM.get((r, cs), []):
                    sub = csub - 3 * cs
                    pt = ps_t.tile([P, P], BF16, tag="pt")
                    nc.tensor.transpose(pt[:], ssb[:, sub * P:(sub + 1) * P], ident[:])
                    sbT = gw.tile([P, P], BF16, tag="sbT")
                    nc.scalar.copy(out=sbT[:], in_=pt[:])
                    g0 = csub * P
                    a = 0
                    while a < min(P, SROWS - g0):
                        cls = (g0 + a) // CB
                        b = min(P, (cls + 1) * CB - g0)
                        nc.scalar.dma_start(
                            out=ss2_c[cls][g0 + a - cls * CB:g0 + b - cls * CB,
                                           r * P:(r + 1) * P],
                            in_=sbT[a:b, :])
                        a = b

            while front_ops:
                front_ops.pop(0)()

        if bisect == 2:
            return

        with (
            tc.tile_pool(name="gath", bufs=2) as gp,
            tc.tile_pool(name="ps3", bufs=2, space="PSUM") as ps3,
            tc.tile_pool(name="dram3", bufs=1, space="DRAM") as dr3,
        ):
            # gathers: one argmax row per partition per (class, q-tile);
            # class-ascending so each starts once its ss2_c writes land
            for c in range(WAY):
                for mq in range(QT):
                    nc.gpsimd.indirect_dma_start(
                        out=ssg[:, mq, c, :], out_offset=None,
                        in_=ss2_c[c][:],
                        in_offset=bass.IndirectOffsetOnAxis(
                            ap=idxl[:, mq, c:c + 1], axis=0))

            if bisect == 3:
                dbg = rp.tile([16, 16], F32)
                nc.vector.memset(dbg[:], 0.0)
                nc.vector.tensor_copy(dbg[:, :WAY], ave2[:16, 0, :])
                nc.sync.dma_start(out=out_v[:], in_=dbg[:])
                return

            ci = dr3.tile([WAY, SPAD], F32)

            # transpose sqrt-distances now: the PE is idle while gathers and
            # cmp/record run; DsT[s-part, qcol] turns the masked sums into
            # tiny matmuls after the collective
            DsT = rp.tile([P, ST, QPAD], BF16)
            for ms in range(ST):
                for mq in range(QT):
                    ptd = ps3.tile([P, P], BF16, tag="ptd")
                    nc.tensor.transpose(
                        ptd[:], Ds[:, mq, ms * P:(ms + 1) * P], ident[:])
                    nc.vector.tensor_copy(out=DsT[:, ms, mq * P:(mq + 1) * P],
                                           in_=ptd[:])

            # cmp in place (DVE), own-class zero + q-tile sums (gpsimd)
            for c in range(WAY):
                for mq in range(QT):
                    nc.vector.tensor_scalar(
                        out=ssg[:, mq, c, :], in0=ssg[:, mq, c, :],
                        scalar1=ave2[:, mq, c:c + 1], scalar2=None,
                        op0=mybir.AluOpType.is_gt)
                nc.vector.tensor_add(out=cmp_sum[:, c, :], in0=ssg[:, 0, c, :],
                                     in1=ssg[:, 1, c, :])
                nc.vector.tensor_add(out=cmp_sum[:, c, :], in0=cmp_sum[:, c, :],
                                     in1=ssg[:, 2, c, :])
                nc.vector.tensor_add(out=cmp_sum[:, c, :], in0=cmp_sum[:, c, :],
                                     in1=ssg[:, 3, c, :])
                nc.vector.memset(cmp_sum[:, c, c * CB:(c + 1) * CB], 0.0)
                for lo, hi in NSPLITS:
                    pr = ps3.tile([1, 384], F32, tag="pr")
                    nc.tensor.matmul(out=pr[:], lhsT=ones_c[:],
                                     rhs=cmp_sum[:, c, lo:hi], start=True, stop=True)
                    prs = gp.tile([1, 384], F32, tag="prs")
                    nc.scalar.copy(out=prs[:], in_=pr[:])
                    nc.sync.dma_start(out=ci[c:c + 1, lo:hi], in_=prs[:])

            co = dr3.tile([WAY, SPAD], F32, addr_space="Shared")
            nc.gpsimd.collective_compute(
                "AllReduce", mybir.AluOpType.add,
                replica_groups=[list(range(N_CORES))],
                ins=[ci[:]], outs=[co[:]])
            recg = rp.tile([8, SPAD], F32)
            nc.sync.dma_start(out=recg[:WAY, :], in_=co[:])

            if bisect == 5:
                dbg = rp.tile([16, 16], F32)
                nc.vector.memset(dbg[:], 0.0)
                nc.vector.tensor_copy(dbg[:WAY, :], recg[:WAY, :16])
                nc.sync.dma_start(out=out_v[:], in_=dbg[:])
                return

            # thr per (class, block); mask; msum
            rsum = rp.tile([8, WAY], F32)
            nzc = rp.tile([8, SPAD], F32)
            nz = rp.tile([8, WAY], F32)
            thr = rp.tile([8, WAY], F32)
            g = recg[:WAY, :SROWS].rearrange("p (b s) -> p b s", b=WAY)
            nc.vector.reduce_sum(rsum[:WAY, :], g, axis=mybir.AxisListType.X)
            nc.vector.tensor_scalar(
                out=nzc[:WAY, :SROWS], in0=recg[:WAY, :SROWS],
                scalar1=0.5, scalar2=None, op0=mybir.AluOpType.is_gt)
            nc.vector.reduce_sum(
                nz[:WAY, :], nzc[:WAY, :SROWS].rearrange("p (b s) -> p b s", b=WAY),
                axis=mybir.AxisListType.X)
            nc.vector.tensor_scalar(
                out=nz[:WAY, :], in0=nz[:WAY, :], scalar1=1.0, scalar2=None,
                op0=mybir.AluOpType.max)
            nc.vector.reciprocal(out=nz[:WAY, :], in_=nz[:WAY, :])
            nc.vector.tensor_mul(out=thr[:WAY, :], in0=rsum[:WAY, :], in1=nz[:WAY, :])

            mask = rp.tile([8, SPAD], BF16)
            nc.vector.memset(mask[:], 0.0)
            for b in range(WAY):
                nc.vector.tensor_scalar(
                    out=mask[:WAY, b * CB:(b + 1) * CB],
                    in0=recg[:WAY, b * CB:(b + 1) * CB],
                    scalar1=thr[:WAY, b:b + 1], scalar2=None,
                    op0=mybir.AluOpType.is_lt)
            msum = rp.tile([8, 1], F32)
            nc.vector.reduce_sum(msum[:WAY, :], mask[:WAY, :SROWS], axis=mybir.AxisListType.X)
            nc.sync.dma_start(out=out_ms[:], in_=msum[:WAY, :])
            if bisect == 6:
                dbg = rp.tile([16, 16], F32)
                nc.vector.memset(dbg[:], 0.0)
                nc.vector.tensor_copy(dbg[:WAY, :], mask[:WAY, :16])
                nc.sync.dma_start(out=out_v[:], in_=dbg[:])
                return

            # masked mean distances: V[:, mq, 5+c] = sum_s DsT[s, q] mask[c, s]
            # mask rows transposed to [s-part, class]; contraction over s via PE
            maskT = rp.tile([P, ST, 8], BF16)
            for ms in range(ST):
                ptm = ps3.tile([P, 8], BF16, tag="ptm", bufs=1)
                nc.tensor.transpose(
                    ptm[:], mask[:, ms * P:(ms + 1) * P], ident[:8, :8])
                nc.scalar.copy(out=maskT[:, ms, :], in_=ptm[:])
            for mq in range(QT):
                pms = ps3.tile([P, 8], F32, tag="pms", bufs=1)
                for ms in range(ST):
                    nc.tensor.matmul(
                        out=pms[:], lhsT=DsT[:, ms, mq * P:(mq + 1) * P],
                        rhs=maskT[:, ms, :], start=(ms == 0), stop=(ms == ST - 1))
                nc.scalar.copy(out=V[:, mq, WAY:WAY + WAY], in_=pms[:, :WAY])

            # group rows by query: out[q, col] = sum_r gmat[r, q] * V[r, col]
            if bisect == 7:
                dbg = rp.tile([16, 16], F32)
                nc.vector.tensor_copy(dbg[:], V[:16, 0, :])
                nc.sync.dma_start(out=out_v[:], in_=dbg[:])
                return
            pv = ps3.tile([16, 16], F32, tag="pv", bufs=1)
            for t in range(QT):
                nc.tensor.matmul(
                    out=pv[:], lhsT=sb_gmat[:, t * 16:(t + 1) * 16], rhs=V[:, t, :],
                    start=(t == 0), stop=(t == QT - 1))
            vsb = rp.tile([16, 16], F32)
            nc.scalar.copy(out=vsb[:], in_=pv[:])
            nc.sync.dma_start(out=out_v[:], in_=vsb[:])


_CACHE = {}


def _get_nc():
    if "nc" not in _CACHE:
        _CACHE["nc"] = _build()
    return _CACHE["nc"]


def kernel(support_set, support_labels, queries, clsW_w, clsW_b):
    support_set = np.asarray(support_set, dtype=np.float32)
    queries = np.asarray(queries, dtype=np.float32)
    W = np.asarray(clsW_w, dtype=np.float32)
    b = np.asarray(clsW_b, dtype=np.float32)
    bf = ml_dtypes.bfloat16
    f8 = ml_dtypes.float8_e4m3

    # weight tiles: wt[m, kp, h, ko, mf] = W[m*128+mf, h*2048+ko*128+kp]
    wt = np.ascontiguousarray(
        (W * WSCALE).reshape(KT, P, 2, KT, P).transpose(0, 4, 2, 3, 1)).astype(f8)
    bias_h = np.ascontiguousarray(b.reshape(KT, P).T)

    # per-core frame matrices
    sup_frames = support_set.reshape(FS, D)
    in_maps = []
    rows = np.arange(QPAD)
    gm = np.zeros((QPAD, 16), np.float32)
    valid_all = rows < QROWS
    gm[valid_all, (rows // T)[valid_all].clip(0, 15)] = 1.0
    gmat_h = np.ascontiguousarray(gm.reshape(QT, P, 16).transpose(1, 0, 2))
    for k in range(N_CORES):
        qids = [(10 * k + j) % NQ for j in range(NQL)]
        qf = queries[qids].reshape(FQ, D)
        frames = np.concatenate([qf, sup_frames], axis=0)  # [350, 2048]
        xth = np.ascontiguousarray(
            frames.T.reshape(KT, P, F).transpose(1, 0, 2)).astype(f8)
        inv = np.where(rows < (QROWS if k < N_CORES - 1 else 5 * T), 0.0, BIG)
        inv = inv.astype(np.float32)
        invb_h = np.ascontiguousarray(inv.reshape(QT, P).T)
        in_maps.append({
            "wt": wt, "xt": xth, "bias": bias_h,
            "invb": invb_h, "gmat": gmat_h,
        })

    nc = _get_nc()
    _CACHE["last_in_maps"] = in_maps
    res = run_bass_kernel_spmd(nc, in_maps, list(range(N_CORES))).results

    msum = np.maximum(res[0]["out_ms"].reshape(WAY), 1.0)
    dist_max = np.zeros((NQ, WAY), np.float32)
    md_raw = np.zeros((NQ, WAY), np.float32)
    for k in range(N_CORES):
        v = res[k]["out_v"]
        for j in range(NQL):
            q = 10 * k + j
            if q >= NQ:
                break
            dist_max[q] = v[j, :WAY] / T
            md_raw[q] = v[j, WAY:2 * WAY]
    contrast = md_raw / (T * (WAY - 1) * msum[None, :])
    logits = dist_max / (contrast + dist_max)
    return dist_max, logits
